# revision 47
# baseline (speedup 1.0000x reference)
"""Banded local-linear layer (nn_LocalLinearLayer) on 8 trn2 NeuronCores.

out[b, o, c] = sum_p W[o, p] * xpad[b, c, p] + bias[o],  band p in [o, o+25)
xpad = pad of x along L: first/last 12 rows block-copied (not reversed).

Strategy (v7, L-sharded, halo tiles, per-chunk tiles):
  - Shard the 4096 output rows across 8 cores (512 rows each); each core
    loads only its slice of the banded weight (~133 KB) and of xpad
    (536 rows incl. halo), with the full free dim B*C = 2048.
  - Per core: 5 output tiles of M=104 rows (last 96). Tile t loads xpad
    rows [104t, 104t+128) (24-row halo duplicated in the DRAM layout) ->
    one K=128 matmul per (tile, 512-col half): 20 matmuls.
  - Every x chunk / output chunk is its OWN SBUF tile object: Tile
    dependency tracking is tile-granular, so shared big tiles serialize
    the pipeline behind unrelated DMA writes.
  - bf16 operands and bf16 output (fp32 PSUM accumulation).
  - Input DMAs on the Sync ring, bias + output DMAs on the Scalar ring.
  - PSUM->SBUF drain (+bias) per 1024-col chunk alternates VectorE
    tensor_scalar_add / ScalarE activation (ACT, the faster one, takes
    the last chunk which sits on the critical tail).
  - A burst of throwaway matmuls on zeroed scratch runs during the DMA
    lead-in to keep the PE pipeline hot from the first real matmul.
"""

import sys

for _p in ("/opt/trn_rl_repo",):
    if _p not in sys.path:
        sys.path.insert(0, _p)

import ml_dtypes
import numpy as np

import concourse.bass as bass
import concourse.tile as tile
from concourse import bacc, mybir
from concourse.bass_utils import run_bass_kernel_spmd

L = 4096
WIN = 25
PAD = 12
PADDED = L + 2 * PAD  # 4120
B = 32
C = 64
NCORES = 8
NF = B * C  # 2048 free dim
RPC = L // NCORES  # 512 output rows per core
M = 104  # output rows per tile (128 - 24 halo)
NT = 5  # tiles per core; last tile has 96 rows / K=120
HALO = WIN - 1  # 24
NCH = 2  # free-dim chunks per tile
NCHUNK = NF // NCH  # 1024
NWARM = 4  # throwaway PE warm-up matmuls

F32 = mybir.dt.float32
BF16 = mybir.dt.bfloat16
NPBF16 = ml_dtypes.bfloat16


def _mt(t):
    return M if t < NT - 1 else RPC - M * (NT - 1)  # 96 for the last tile


def _kt(t):
    return min(128, RPC + HALO - M * t)  # 120 for the last tile


def _build_nc():
    nc = bacc.Bacc("TRN2", target_bir_lowering=False, debug=False, num_devices=NCORES)
    xm_d = nc.dram_tensor("xm", [128, NT, NF], BF16, kind="ExternalInput").ap()
    wa_d = nc.dram_tensor("wa", [128, NT, M], BF16, kind="ExternalInput").ap()
    bias_d = nc.dram_tensor("bias", [M, NT], F32, kind="ExternalInput").ap()
    out_d = nc.dram_tensor("out", [M, NT, NF], BF16, kind="ExternalOutput").ap()

    with tile.TileContext(nc) as tc:
        with (
            tc.tile_pool(name="main", bufs=1) as pool,
            tc.tile_pool(name="ps", bufs=6, space=bass.MemorySpace.PSUM) as pspool,
            tc.tile_pool(name="psw", bufs=1, space=bass.MemorySpace.PSUM) as pswarm,
        ):
            wa0_s = pool.tile([128, M], BF16)
            war_s = pool.tile([128, NT - 1, M], BF16)
            bias_s = pool.tile([M, NT], F32)
            warm_s = pool.tile([128, 512], BF16)
            # tile 0's x arrives in two 1024-col pieces so the pipeline
            # starts as early as possible; later tiles are one DMA each
            # (each HWDGE DMA costs ~600ns of ring issue time regardless
            # of payload, so fewer+bigger transfers win)
            xs0 = [pool.tile([128, NCHUNK], BF16, name=f"x0_{j}") for j in range(2)]
            xs = [
                pool.tile([_kt(t), NF], BF16, name=f"x{t}") for t in range(1, NT)
            ]
            # one extra (never-DMAed) column: a dummy write into it that
            # reads the last x tile holds the chunk's out-DMA back until
            # the whole input stream has landed (see below)
            os_ = [
                [
                    pool.tile([_mt(t), NCHUNK + 1], BF16, name=f"o{t}_{j}")
                    for j in range(NCH)
                ]
                for t in range(NT)
            ]

            # PE warm-up: harmless matmuls on zeroed scratch while input
            # DMAs stream in, so the PE pipeline is hot for real work
            nc.gpsimd.memset(warm_s[:], 0)
            for _ in range(NWARM):
                pz = pswarm.tile([128, 512], F32)
                nc.tensor.matmul(
                    pz[:], warm_s[:, :128], warm_s[:], start=True, stop=True
                )

            nc.scalar.dma_start(bias_s[:], bias_d)
            # input order: tile-0 weights + x first so the first matmul's
            # semaphores complete as early as possible (DMA completion
            # trails data by ~2-3 us of fixed pipeline latency). Tiles 3-4
            # load on the Scalar ring: both rings drain concurrently, so
            # the late tiles' completion semaphores arrive much earlier.
            nc.sync.dma_start(wa0_s[:], wa_d[:, 0])
            nc.sync.dma_start(xs0[0][:], xm_d[:, 0, :NCHUNK])
            nc.sync.dma_start(xs0[1][:], xm_d[:, 0, NCHUNK:])
            nc.sync.dma_start(xs[0][:], xm_d[: _kt(1), 1, :])
            nc.sync.dma_start(war_s[:], wa_d[:, 1:])
            for t in range(2, NT):
                nc.sync.dma_start(xs[t - 1][:], xm_d[: _kt(t), t, :])

            chunk = 0  # global 1024-col chunk counter for drain-engine parity
            for t in range(NT):
                mt, kt = _mt(t), _kt(t)
                lhsT = wa0_s[:kt, :mt] if t == 0 else war_s[:kt, t - 1, :mt]
                for j in range(NCH):
                    # one matmul + one drain per 512-col half (a matmul's
                    # PSUM output may not cross a 2 KB bank boundary, and
                    # per-half PSUM tiles recycle sooner); both halves of a
                    # chunk drain on the SAME engine (alternating per chunk)
                    # so each output tile has a single writer. Out-DMAs are
                    # issued from the otherwise-idle Sync engine (a dma_start
                    # occupies its issuing engine ~0.7us, which would starve
                    # the drains)
                    pss = []
                    for h in range(2):
                        hs = j * NCHUNK + h * 512
                        rhs = (
                            xs0[j][:, h * 512 : (h + 1) * 512]
                            if t == 0
                            else xs[t - 1][:, hs : hs + 512]
                        )
                        ps = pspool.tile([M, 512], F32)
                        pss.append(ps)
                        nc.tensor.matmul(
                            ps[:mt], lhsT, rhs, start=True, stop=True
                        )
                    if t < 3:
                        # hold this chunk's out-DMA until the LAST x tile has
                        # fully landed: out data otherwise round-robins with
                        # still-pending x transfers on the SDMA engines and
                        # starves their completion semaphores (observed: x4's
                        # sems at ~20us for an 11us issue). The dummy write
                        # (same engine as the drains -> no cross-engine tile
                        # writers) taints the os tile so the DMA below waits.
                        # It comes BEFORE the drains so only the first dummy
                        # per engine actually waits; later ones are free.
                        dsl = os_[t][j][:1, NCHUNK : NCHUNK + 1]
                        if chunk % 2 == 1:
                            nc.scalar.activation(
                                dsl,
                                xs[NT - 2][:1, :1],
                                mybir.ActivationFunctionType.Identity,
                                bias=bias_s[:1, t : t + 1],
                            )
                        else:
                            nc.vector.tensor_scalar_add(
                                dsl, xs[NT - 2][:1, :1], bias_s[:1, t : t + 1]
                            )
                    for h in range(2):
                        osl = os_[t][j][:, h * 512 : (h + 1) * 512]
                        if chunk % 2 == 1:
                            nc.scalar.activation(
                                osl,
                                pss[h][:mt],
                                mybir.ActivationFunctionType.Identity,
                                bias=bias_s[:mt, t : t + 1],
                            )
                        else:
                            nc.vector.tensor_scalar_add(
                                osl, pss[h][:mt], bias_s[:mt, t : t + 1]
                            )
                    sl = slice(j * NCHUNK, (j + 1) * NCHUNK)
                    nc.sync.dma_start(
                        out_d[:mt, t, sl], os_[t][j][:, :NCHUNK]
                    )
                    chunk += 1

    nc.compile()
    return nc


_NC = None


def _get_nc():
    global _NC
    if _NC is None:
        _NC = _build_nc()
    return _NC


# band mask within a [104 rows, 128 cols] weight block: col k nonzero for
# row m iff k in [m, m+WIN)  (same for every tile/core)
_K = np.arange(128)[None, :]
_MM = np.arange(M)[:, None]
_BMASK = ((_K >= _MM) & (_K < _MM + WIN)).astype(np.float32)


def _make_in_maps(x, W, b):
    x = np.asarray(x, dtype=np.float32)
    W = np.asarray(W, dtype=np.float32)
    b = np.asarray(b, dtype=np.float32)
    xl = np.ascontiguousarray(x.transpose(1, 0, 2)).reshape(L, NF)
    xpad = np.concatenate([xl[:PAD], xl, xl[-PAD:]], 0).astype(NPBF16)  # [4120,NF]

    in_maps = []
    for c in range(NCORES):
        r0 = RPC * c
        xm = np.zeros((128, NT, NF), NPBF16)
        wa = np.zeros((128, NT, M), NPBF16)
        bias = np.zeros((M, NT), np.float32)
        for t in range(NT):
            mt, kt = _mt(t), _kt(t)
            o0 = r0 + M * t
            xm[:kt, t] = xpad[o0 : o0 + kt]
            blk = W[o0 : o0 + mt, o0 : o0 + kt] * _BMASK[:mt, :kt]
            wa[:kt, t, :mt] = blk.T
            bias[:mt, t] = b[o0 : o0 + mt]
        in_maps.append({"xm": xm, "wa": wa, "bias": bias})
    return in_maps


def _gather_core(out_arr):
    """[104, NT, 2048] bf16 -> [512, B, C] f32 rows for one core."""
    rows = np.empty((RPC, B, C), np.float32)
    for t in range(NT):
        mt = _mt(t)
        rows[M * t : M * t + mt] = (
            out_arr[:mt, t].astype(np.float32).reshape(mt, B, C)
        )
    return rows


def _gather(results):
    rows = np.concatenate(
        [_gather_core(np.asarray(r["out"])) for r in results], axis=0
    )  # [L, B, C]
    return np.ascontiguousarray(rows.transpose(1, 0, 2))


def kernel(x: np.ndarray, W: np.ndarray, b: np.ndarray) -> np.ndarray:
    nc = _get_nc()
    res = run_bass_kernel_spmd(nc, _make_in_maps(x, W, b), list(range(NCORES)))
    return _gather(res.results)


if __name__ == "__main__":
    rng = np.random.default_rng(0)
    x = rng.standard_normal((B, L, C), dtype=np.float32)
    W = rng.standard_normal((L, PADDED), dtype=np.float32) * 0.02
    b = rng.standard_normal((L,), dtype=np.float32) * 0.02
    print(kernel(x, W, b).shape)


# revision 50
# speedup vs baseline: 1.1100x; 1.1100x over previous
"""Banded local-linear layer (nn_LocalLinearLayer) on 8 trn2 NeuronCores.

out[b, o, c] = sum_p W[o, p] * xpad[b, c, p] + bias[o],  band p in [o, o+25)
xpad = pad of x along L: first/last 12 rows block-copied (not reversed).

Strategy (v7, L-sharded, halo tiles, per-chunk tiles):
  - Shard the 4096 output rows across 8 cores (512 rows each); each core
    loads only its slice of the banded weight (~133 KB) and of xpad
    (536 rows incl. halo), with the full free dim B*C = 2048.
  - Per core: 5 output tiles of M=104 rows (last 96). Tile t loads xpad
    rows [104t, 104t+128) (24-row halo duplicated in the DRAM layout) ->
    one K=128 matmul per (tile, 512-col half): 20 matmuls.
  - Every x chunk / output chunk is its OWN SBUF tile object: Tile
    dependency tracking is tile-granular, so shared big tiles serialize
    the pipeline behind unrelated DMA writes.
  - bf16 operands and bf16 output (fp32 PSUM accumulation).
  - Input DMAs on the Sync ring, bias + output DMAs on the Scalar ring.
  - PSUM->SBUF drain (+bias) per 1024-col chunk alternates VectorE
    tensor_scalar_add / ScalarE activation (ACT, the faster one, takes
    the last chunk which sits on the critical tail).
  - A burst of throwaway matmuls on zeroed scratch runs during the DMA
    lead-in to keep the PE pipeline hot from the first real matmul.
"""

import sys

for _p in ("/opt/trn_rl_repo",):
    if _p not in sys.path:
        sys.path.insert(0, _p)

import ml_dtypes
import numpy as np

import concourse.bass as bass
import concourse.tile as tile
from concourse import bacc, mybir
from concourse.bass_utils import run_bass_kernel_spmd

L = 4096
WIN = 25
PAD = 12
PADDED = L + 2 * PAD  # 4120
B = 32
C = 64
NCORES = 8
NF = B * C  # 2048 free dim
RPC = L // NCORES  # 512 output rows per core
M = 104  # output rows per tile (128 - 24 halo)
NT = 5  # tiles per core; last tile has 96 rows / K=120
HALO = WIN - 1  # 24
NCH = 2  # free-dim chunks per tile
NCHUNK = NF // NCH  # 1024
NWARM = 4  # throwaway PE warm-up matmuls

F32 = mybir.dt.float32
BF16 = mybir.dt.bfloat16
NPBF16 = ml_dtypes.bfloat16


def _mt(t):
    return M if t < NT - 1 else RPC - M * (NT - 1)  # 96 for the last tile


def _kt(t):
    return min(128, RPC + HALO - M * t)  # 120 for the last tile


def _build_nc():
    nc = bacc.Bacc("TRN2", target_bir_lowering=False, debug=False, num_devices=NCORES)
    xm_d = nc.dram_tensor("xm", [128, NT, NF], BF16, kind="ExternalInput").ap()
    wa_d = nc.dram_tensor("wa", [128, NT, M], BF16, kind="ExternalInput").ap()
    bias_d = nc.dram_tensor("bias", [M, NT], F32, kind="ExternalInput").ap()
    out_d = nc.dram_tensor("out", [M, NT, NF], BF16, kind="ExternalOutput").ap()

    with tile.TileContext(nc) as tc:
        with (
            tc.tile_pool(name="main", bufs=1) as pool,
            tc.tile_pool(name="ps", bufs=6, space=bass.MemorySpace.PSUM) as pspool,
            tc.tile_pool(name="psw", bufs=1, space=bass.MemorySpace.PSUM) as pswarm,
        ):
            wa0_s = pool.tile([128, M], BF16)
            war_s = pool.tile([128, NT - 1, M], BF16)
            bias_s = pool.tile([M, NT], F32)
            warm_s = pool.tile([128, 512], BF16)
            # tile 0's x arrives in two 1024-col pieces so the pipeline
            # starts as early as possible; later tiles are one DMA each
            # (each HWDGE DMA costs ~600ns of ring issue time regardless
            # of payload, so fewer+bigger transfers win)
            xs0 = [pool.tile([128, NCHUNK], BF16, name=f"x0_{j}") for j in range(2)]
            xs = [
                pool.tile([_kt(t), NF], BF16, name=f"x{t}") for t in range(1, NT)
            ]
            os_ = [
                [
                    pool.tile([_mt(t), NCHUNK], BF16, name=f"o{t}_{j}")
                    for j in range(NCH)
                ]
                for t in range(NT)
            ]

            # PE warm-up: harmless matmuls on zeroed scratch while input
            # DMAs stream in, so the PE pipeline is hot for real work
            nc.gpsimd.memset(warm_s[:], 0)
            for _ in range(NWARM):
                pz = pswarm.tile([128, 512], F32)
                nc.tensor.matmul(
                    pz[:], warm_s[:, :128], warm_s[:], start=True, stop=True
                )

            nc.scalar.dma_start(bias_s[:], bias_d)
            # input order: tile-0 weights + x first so the first matmul's
            # semaphores complete as early as possible (DMA completion
            # trails data by ~2-3 us of fixed pipeline latency). Tiles 3-4
            # load on the Scalar ring: both rings drain concurrently, so
            # the late tiles' completion semaphores arrive much earlier.
            nc.sync.dma_start(wa0_s[:], wa_d[:, 0])
            nc.sync.dma_start(xs0[0][:], xm_d[:, 0, :NCHUNK])
            nc.sync.dma_start(xs0[1][:], xm_d[:, 0, NCHUNK:])
            nc.sync.dma_start(xs[0][:], xm_d[: _kt(1), 1, :])
            nc.sync.dma_start(war_s[:], wa_d[:, 1:])
            for t in range(2, NT):
                nc.sync.dma_start(xs[t - 1][:], xm_d[: _kt(t), t, :])

            chunk = 0  # global 1024-col chunk counter for drain-engine parity
            for t in range(NT):
                mt, kt = _mt(t), _kt(t)
                lhsT = wa0_s[:kt, :mt] if t == 0 else war_s[:kt, t - 1, :mt]
                for j in range(NCH):
                    # one matmul + one drain per 512-col half (a matmul's
                    # PSUM output may not cross a 2 KB bank boundary, and
                    # per-half PSUM tiles recycle sooner); both halves of a
                    # chunk drain on the SAME engine (alternating per chunk)
                    # so each output tile has a single writer. Out-DMAs are
                    # issued from the otherwise-idle Sync engine (a dma_start
                    # occupies its issuing engine ~0.7us, which would starve
                    # the drains)
                    pss = []
                    for h in range(2):
                        hs = j * NCHUNK + h * 512
                        rhs = (
                            xs0[j][:, h * 512 : (h + 1) * 512]
                            if t == 0
                            else xs[t - 1][:, hs : hs + 512]
                        )
                        ps = pspool.tile([M, 512], F32)
                        pss.append(ps)
                        nc.tensor.matmul(
                            ps[:mt], lhsT, rhs, start=True, stop=True
                        )
                    for h in range(2):
                        osl = os_[t][j][:, h * 512 : (h + 1) * 512]
                        if chunk % 2 == 1:
                            nc.scalar.activation(
                                osl,
                                pss[h][:mt],
                                mybir.ActivationFunctionType.Identity,
                                bias=bias_s[:mt, t : t + 1],
                            )
                        else:
                            nc.vector.tensor_scalar_add(
                                osl, pss[h][:mt], bias_s[:mt, t : t + 1]
                            )
                    sl = slice(j * NCHUNK, (j + 1) * NCHUNK)
                    nc.sync.dma_start(out_d[:mt, t, sl], os_[t][j][:])
                    chunk += 1

    nc.compile()
    return nc


_NC = None


def _get_nc():
    global _NC
    if _NC is None:
        _NC = _build_nc()
    return _NC


# band mask within a [104 rows, 128 cols] weight block: col k nonzero for
# row m iff k in [m, m+WIN)  (same for every tile/core)
_K = np.arange(128)[None, :]
_MM = np.arange(M)[:, None]
_BMASK = ((_K >= _MM) & (_K < _MM + WIN)).astype(np.float32)


def _make_in_maps(x, W, b):
    x = np.asarray(x, dtype=np.float32)
    W = np.asarray(W, dtype=np.float32)
    b = np.asarray(b, dtype=np.float32)
    xl = np.ascontiguousarray(x.transpose(1, 0, 2)).reshape(L, NF)
    xpad = np.concatenate([xl[:PAD], xl, xl[-PAD:]], 0).astype(NPBF16)  # [4120,NF]

    in_maps = []
    for c in range(NCORES):
        r0 = RPC * c
        xm = np.zeros((128, NT, NF), NPBF16)
        wa = np.zeros((128, NT, M), NPBF16)
        bias = np.zeros((M, NT), np.float32)
        for t in range(NT):
            mt, kt = _mt(t), _kt(t)
            o0 = r0 + M * t
            xm[:kt, t] = xpad[o0 : o0 + kt]
            blk = W[o0 : o0 + mt, o0 : o0 + kt] * _BMASK[:mt, :kt]
            wa[:kt, t, :mt] = blk.T
            bias[:mt, t] = b[o0 : o0 + mt]
        in_maps.append({"xm": xm, "wa": wa, "bias": bias})
    return in_maps


def _gather_core(out_arr):
    """[104, NT, 2048] bf16 -> [512, B, C] f32 rows for one core."""
    rows = np.empty((RPC, B, C), np.float32)
    for t in range(NT):
        mt = _mt(t)
        rows[M * t : M * t + mt] = (
            out_arr[:mt, t].astype(np.float32).reshape(mt, B, C)
        )
    return rows


def _gather(results):
    rows = np.concatenate(
        [_gather_core(np.asarray(r["out"])) for r in results], axis=0
    )  # [L, B, C]
    return np.ascontiguousarray(rows.transpose(1, 0, 2))


def kernel(x: np.ndarray, W: np.ndarray, b: np.ndarray) -> np.ndarray:
    nc = _get_nc()
    res = run_bass_kernel_spmd(nc, _make_in_maps(x, W, b), list(range(NCORES)))
    return _gather(res.results)


if __name__ == "__main__":
    rng = np.random.default_rng(0)
    x = rng.standard_normal((B, L, C), dtype=np.float32)
    W = rng.standard_normal((L, PADDED), dtype=np.float32) * 0.02
    b = rng.standard_normal((L,), dtype=np.float32) * 0.02
    print(kernel(x, W, b).shape)
